# revision 13
# baseline (speedup 1.0000x reference)
"""Bahdanau-style additive cross-attention on 8 Trainium2 NeuronCores.

Math: scores[b,q,k] = sum_h Wv[h] * tanh(qp[b,q,h] + 2*cp[b,k,h]) + bv
      out = softmax_k(scores) @ context

The tanh over the [B,Lq,Lk,H] intermediate (134M elements) is the whole
cost.  We approximate tanh(s) on |s| <= 9.2 by a truncated sine series
    tanh(s) ~= sum_r g_r * sin(w_r * s),   w_r = (r+1)*pi/L
which separates:  sin(w(a+b)) = sin(wa)cos(wb) + cos(wa)sin(wb).
So scores becomes 2R rank-128 matmuls over bf16 sin/cos factor tiles --
tensor-engine work instead of 16.8M scalar-engine tanh evals per core.

Per-frequency factors need range reduction for ACT Sin (accurate only on
[-pi, pi]): y = w*x/2pi + c, n = round(y) (int32-cast), f = y - n in
[-0.5, 0.5], then sin = Sin(2pi*f) and cos = Sin(pi/2 - 2pi*|f|)
with |f| via sign-bit masking.

Sharding: core i handles batch b = i//2, query half qh = i%2
(256 queries x 512 keys x 128 hidden per core).
"""

import os
import sys

import numpy as np

for _p in ("/opt/trn_rl_repo", "/root/.axon_site/_ro/trn_rl_repo"):
    if os.path.isdir(_p) and _p not in sys.path:
        sys.path.insert(0, _p)

import concourse.bass as bass  # noqa: E402
import concourse.mybir as mybir  # noqa: E402
import concourse.tile as tile  # noqa: E402
from concourse import bacc  # noqa: E402
from concourse.bass_utils import run_bass_kernel_spmd  # noqa: E402

AF = mybir.ActivationFunctionType
AO = mybir.AluOpType
dt = mybir.dt

# problem shape (hardcoded per contract)
B, LQ, LK = 4, 512, 512
DQ, DC, H = 256, 256, 128
N_CORES = 8
LQ_C = LQ // 2  # 256 queries per core

# sine-series fit parameters
R = 12          # number of frequencies
L = 11.0        # half-period
SMAX = 9.2      # fit domain |s| <= SMAX
K_OFF = 16.0    # positivity offset (integer) folded into prescale consts
RC = 3          # frequencies per pipeline chunk
N_CHUNKS = R // RC

TWO_PI = float(2.0 * np.pi)
HALF_PI = float(np.pi / 2.0)


def _fit_gamma() -> np.ndarray:
    s = np.linspace(0.0, SMAX, 4001)
    A = np.stack([np.sin((r + 1) * np.pi / L * s) for r in range(R)], axis=1)
    g, *_ = np.linalg.lstsq(A, np.tanh(s), rcond=None)
    return g.astype(np.float64)


_GAMMA = _fit_gamma()

_prog_cache = {}


def _register_const(nc, value, dtype=dt.float32):
    t = nc.alloc_sbuf_tensor(f"constx-{value}", [128, 1], dtype)
    nc.gpsimd.memset(t.ap(), value)
    nc.const_aps.aps[(dtype, value)] = t.ap()


def _build_program():
    nc = bacc.Bacc(None, target_bir_lowering=False)
    _register_const(nc, HALF_PI)

    # per-core inputs (host pre-transposed / pre-sharded)
    qT = nc.dram_tensor("qT", [DQ, LQ_C], dt.float32, kind="ExternalInput")
    cT = nc.dram_tensor("cT", [DC, LK], dt.float32, kind="ExternalInput")
    ctx = nc.dram_tensor("ctx", [LK, DC + 2], dt.float32, kind="ExternalInput")
    wqT = nc.dram_tensor("wqT", [DQ, H], dt.float32, kind="ExternalInput")
    wcT = nc.dram_tensor("wcT", [DC, H], dt.float32, kind="ExternalInput")
    ca = nc.dram_tensor("ca", [H, R], dt.float32, kind="ExternalInput")
    cb = nc.dram_tensor("cb", [H, R], dt.float32, kind="ExternalInput")
    gwv = nc.dram_tensor("gwv", [H, R], dt.float32, kind="ExternalInput")
    out = nc.dram_tensor("out", [LQ_C, DC], dt.float32, kind="ExternalOutput")

    FA = RC * LQ_C   # a-side chunk free size
    FB = RC * LK     # b-side chunk free size

    with tile.TileContext(nc) as tc:
        with (
            tc.tile_pool(name="sb", bufs=1) as sb,
            tc.tile_pool(name="w1", bufs=2) as w1,
            tc.tile_pool(name="w2", bufs=3) as w2,
            tc.tile_pool(name="ps_a", bufs=2, space="PSUM") as ps_a,
            tc.tile_pool(name="ps_s", bufs=4, space="PSUM") as ps_s,
        ):
            # ---- DMA in: projection operands first, ctx last ----
            wq_t = []
            for i in range(2):
                t = sb.tile([128, H], dt.float32, tag=f"wq{i}", name=f"wq{i}")
                nc.sync.dma_start(t[:], wqT[i * 128:(i + 1) * 128, :])
                wq_t.append(t)
            wc_t = []
            for i in range(2):
                t = sb.tile([128, H], dt.float32, tag=f"wc{i}", name=f"wc{i}")
                nc.sync.dma_start(t[:], wcT[i * 128:(i + 1) * 128, :])
                wc_t.append(t)
            qT_t = []
            for i in range(2):
                t = sb.tile([128, LQ_C], dt.float32, tag=f"qT{i}", name=f"qT{i}")
                nc.sync.dma_start(t[:], qT[i * 128:(i + 1) * 128, :])
                qT_t.append(t)
            cT_t = []
            for i in range(2):
                t = sb.tile([128, LK], dt.float32, tag=f"cT{i}", name=f"cT{i}")
                nc.sync.dma_start(t[:], cT[i * 128:(i + 1) * 128, :])
                cT_t.append(t)
            ca_t = sb.tile([H, R], dt.float32, tag="ca")
            nc.sync.dma_start(ca_t[:], ca[:])
            cb_t = sb.tile([H, R], dt.float32, tag="cb")
            nc.sync.dma_start(cb_t[:], cb[:])
            gwv_t = sb.tile([H, R], dt.float32, tag="gwv")
            nc.sync.dma_start(gwv_t[:], gwv[:])
            # ctx is only needed for the final attn @ context matmul
            ctx_t = []
            for i in range(4):
                t = sb.tile([128, DC + 2], dt.float32, tag=f"ctx{i}",
                            name=f"ctx{i}")
                nc.gpsimd.dma_start(t[:], ctx[i * 128:(i + 1) * 128, :])
                ctx_t.append(t)

            # ---- projections on PE (fp32): qpT[h,q], cpT[h,k] ----
            qpT_ps = ps_a.tile([H, LQ_C], dt.float32, tag="psA")
            for i in range(2):
                nc.tensor.matmul(qpT_ps[:], wq_t[i][:], qT_t[i][:],
                                 start=(i == 0), stop=(i == 1))
            cpT_ps = ps_a.tile([H, LK], dt.float32, tag="psA")
            for i in range(2):
                nc.tensor.matmul(cpT_ps[:], wc_t[i][:], cT_t[i][:],
                                 start=(i == 0), stop=(i == 1))
            qpTs = sb.tile([H, LQ_C], dt.float32, tag="qpTs")
            nc.vector.tensor_copy(qpTs[:], qpT_ps[:])
            cpTs = sb.tile([H, LK], dt.float32, tag="cpTs")
            nc.scalar.copy(cpTs[:], cpT_ps[:])

            # ---- scoresT PSUM tiles (accumulated across all chunks) ----
            scoresT = [ps_s.tile([128, LQ_C], dt.float32, tag="psS",
                                 name=f"scoresT{kt}")
                       for kt in range(4)]

            # ---- per-chunk factor pipeline + matmuls ----
            for ch in range(N_CHUNKS):
                r0 = ch * RC
                # prescale: y = x*(w_r/2pi) + table  (DVE)
                Ya = w1.tile([H, FA], dt.float32, tag="Ya", name=f"Ya{ch}")
                Yb = w1.tile([H, FB], dt.float32, tag="Yb", name=f"Yb{ch}")
                for j in range(RC):
                    r = r0 + j
                    nc.vector.tensor_scalar(
                        Ya[:, j * LQ_C:(j + 1) * LQ_C], qpTs[:],
                        float((r + 1) / (2 * L)), ca_t[:, r:r + 1],
                        AO.mult, AO.add)
                for j in range(RC):
                    r = r0 + j
                    nc.vector.tensor_scalar(
                        Yb[:, j * LK:(j + 1) * LK], cpTs[:],
                        float((r + 1) / L), cb_t[:, r:r + 1],
                        AO.mult, AO.add)
                # n = round-to-nearest(y): int32 cast on GPSIMD
                Na = w1.tile([H, FA], dt.int32, tag="Na", name=f"Na{ch}")
                nc.gpsimd.tensor_copy(Na[:], Ya[:])
                Nb = w1.tile([H, FB], dt.int32, tag="Nb", name=f"Nb{ch}")
                nc.gpsimd.tensor_copy(Nb[:], Yb[:])
                # f = y - n in [-0.5, 0.5]  (DVE)
                Fa = w1.tile([H, FA], dt.float32, tag="Fa", name=f"Fa{ch}")
                nc.vector.tensor_tensor(Fa[:], Ya[:], Na[:], AO.subtract)
                Fb = w1.tile([H, FB], dt.float32, tag="Fb", name=f"Fb{ch}")
                nc.vector.tensor_tensor(Fb[:], Yb[:], Nb[:], AO.subtract)
                # |f| via sign-bit mask  (DVE)
                Ga = w1.tile([H, FA], dt.float32, tag="Ga", name=f"Ga{ch}")
                nc.vector.tensor_scalar(
                    Ga[:].bitcast(dt.int32), Fa[:].bitcast(dt.int32),
                    0x7FFFFFFF, None, AO.bitwise_and)
                Gb = w1.tile([H, FB], dt.float32, tag="Gb", name=f"Gb{ch}")
                nc.vector.tensor_scalar(
                    Gb[:].bitcast(dt.int32), Fb[:].bitcast(dt.int32),
                    0x7FFFFFFF, None, AO.bitwise_and)
                # sin = Sin(2pi f); cos = Sin(pi/2 - 2pi|f|)  (ACT, bf16 out)
                sinA = w1.tile([H, FA], dt.bfloat16, tag="sinA", name=f"sinA{ch}")
                nc.scalar.activation(sinA[:], Fa[:], AF.Sin, scale=TWO_PI)
                cosA = w1.tile([H, FA], dt.bfloat16, tag="cosA", name=f"cosA{ch}")
                nc.scalar.activation(cosA[:], Ga[:], AF.Sin,
                                     bias=HALF_PI, scale=-TWO_PI)
                sinB = w2.tile([H, FB], dt.bfloat16, tag="sinB", name=f"sinB{ch}")
                nc.scalar.activation(sinB[:], Fb[:], AF.Sin, scale=TWO_PI)
                cosB = w2.tile([H, FB], dt.bfloat16, tag="cosB", name=f"cosB{ch}")
                nc.scalar.activation(cosB[:], Gb[:], AF.Sin,
                                     bias=HALF_PI, scale=-TWO_PI)
                # A-side *= g_r * Wv[h]  (DVE, bf16)
                WsinA = w2.tile([H, FA], dt.bfloat16, tag="WsinA",
                                name=f"WsinA{ch}")
                WcosA = w2.tile([H, FA], dt.bfloat16, tag="WcosA",
                                name=f"WcosA{ch}")
                for j in range(RC):
                    r = r0 + j
                    nc.vector.tensor_scalar(
                        WsinA[:, j * LQ_C:(j + 1) * LQ_C],
                        sinA[:, j * LQ_C:(j + 1) * LQ_C],
                        gwv_t[:, r:r + 1], None, AO.mult)
                    nc.vector.tensor_scalar(
                        WcosA[:, j * LQ_C:(j + 1) * LQ_C],
                        cosA[:, j * LQ_C:(j + 1) * LQ_C],
                        gwv_t[:, r:r + 1], None, AO.mult)
                # scoresT[k,q] += cosB_r^T @ WsinA_r + sinB_r^T @ WcosA_r
                first = (ch == 0)
                last = (ch == N_CHUNKS - 1)
                for kt in range(4):
                    for j in range(RC):
                        ksl = slice(j * LK + kt * 128, j * LK + kt * 128 + 128)
                        qsl = slice(j * LQ_C, (j + 1) * LQ_C)
                        nc.tensor.matmul(
                            scoresT[kt][:], cosB[:, ksl], WsinA[:, qsl],
                            start=(first and j == 0), stop=False)
                        nc.tensor.matmul(
                            scoresT[kt][:], sinB[:, ksl], WcosA[:, qsl],
                            start=False, stop=(last and j == RC - 1))

            # ---- softmax numerator: E = exp(scoresT), float32r for fast
            # final matmuls (no max-sub needed: |scores| <= sum|Wv| ~ 3.3) ----
            E_t = []
            for kt in range(4):
                e = sb.tile([128, LQ_C], dt.float32r, tag=f"E{kt}", name=f"E{kt}")
                nc.scalar.activation(e[:], scoresT[kt][:], AF.Exp)
                E_t.append(e)
            # float32r copies of context (with appended ones column:
            # column DC of the output accumulates S = sum_k E)
            ctxr_t = []
            for kt in range(4):
                t = sb.tile([128, DC + 2], dt.float32r, tag=f"ctxr{kt}",
                            name=f"ctxr{kt}")
                nc.vector.tensor_copy(t[:], ctx_t[kt][:])
                ctxr_t.append(t)

            for qt in range(2):
                qsl = slice(qt * 128, (qt + 1) * 128)
                o_ps = ps_a.tile([128, DC + 2], dt.float32, tag="psA",
                                 name=f"ops{qt}")
                for kt in range(4):
                    nc.tensor.matmul(o_ps[:], E_t[kt][:, qsl], ctxr_t[kt][:],
                                     start=(kt == 0), stop=(kt == 3))
                rec = sb.tile([128, 1], dt.float32, tag=f"rec{qt}",
                              name=f"rec{qt}")
                nc.vector.reciprocal(rec[:], o_ps[:, DC:DC + 1])
                outF = sb.tile([128, DC], dt.float32, tag=f"outF{qt}",
                               name=f"outF{qt}")
                nc.vector.tensor_scalar(outF[:], o_ps[:, :DC], rec[:], None,
                                        AO.mult)
                nc.sync.dma_start(out[qt * 128:(qt + 1) * 128, :], outF[:])

    if not nc.is_finalized():
        nc.finalize()
    return nc


def _host_tables(Wq, bq, Wc, bc, Wv):
    rr = np.arange(1, R + 1, dtype=np.float64)
    ca = (rr[None, :] * bq.astype(np.float64)[:, None] / (2 * L)) + K_OFF
    cb = (rr[None, :] * bc.astype(np.float64)[:, None] / L) + K_OFF
    gwv = _GAMMA[None, :] * Wv[0].astype(np.float64)[:, None]
    return (ca.astype(np.float32), cb.astype(np.float32),
            gwv.astype(np.float32))


def kernel(**inputs):
    query = np.ascontiguousarray(np.asarray(inputs["query"], np.float32))
    context = np.ascontiguousarray(np.asarray(inputs["context"], np.float32))
    Wq = np.asarray(inputs["Wq"], np.float32)
    bq = np.asarray(inputs["bq"], np.float32)
    Wc = np.asarray(inputs["Wc"], np.float32)
    bc = np.asarray(inputs["bc"], np.float32)
    Wv = np.asarray(inputs["Wv"], np.float32)

    if "prog" not in _prog_cache:
        _prog_cache["prog"] = _build_program()
    nc = _prog_cache["prog"]

    ca, cb, gwv = _host_tables(Wq, bq, Wc, bc, Wv)
    wqT = np.ascontiguousarray(Wq.T)  # [DQ, H]
    wcT = np.ascontiguousarray(Wc.T)  # [DC, H]

    in_maps = []
    for core in range(N_CORES):
        b, qh = core // 2, core % 2
        qTv = np.ascontiguousarray(
            query[b].T[:, qh * LQ_C:(qh + 1) * LQ_C])  # [DQ, LQ_C]
        cTv = np.ascontiguousarray(context[b].T)       # [DC, LK]
        ctx_aug = np.concatenate(
            [context[b], np.ones((LK, 1), np.float32), np.zeros((LK, 1), np.float32)], axis=1)
        in_maps.append({
            "qT": qTv, "cT": cTv, "ctx": np.ascontiguousarray(ctx_aug),
            "wqT": wqT, "wcT": wcT, "ca": ca, "cb": cb, "gwv": gwv,
        })

    try:
        res = run_bass_kernel_spmd(nc, in_maps, list(range(N_CORES)))
        per_core = [res.results[c]["out"] for c in range(N_CORES)]
    except Exception:
        # Transient NRT device crashes have been observed; the in-process
        # PJRT client can be left unusable, so retry in a fresh interpreter.
        per_core = _run_in_subprocess(in_maps)

    out = np.empty((B, LQ, DC), np.float32)
    for core in range(N_CORES):
        b, qh = core // 2, core % 2
        out[b, qh * LQ_C:(qh + 1) * LQ_C, :] = per_core[core]
    return out


def _run_in_subprocess(in_maps):
    import subprocess
    import tempfile
    import time

    tmp = tempfile.mkdtemp()
    inp = os.path.join(tmp, "in.npz")
    outp = os.path.join(tmp, "out.npz")
    flat = {}
    for c, m in enumerate(in_maps):
        for k, v in m.items():
            flat[f"{k}__{c}"] = v
    np.savez(inp, **flat)
    code = (
        "import sys, numpy as np\n"
        f"sys.path.insert(0, {os.path.dirname(os.path.abspath(__file__))!r})\n"
        "import kernel as KK\n"
        f"d = np.load({inp!r})\n"
        f"in_maps = [{{k.rsplit('__', 1)[0]: d[k] for k in d.files "
        f"if k.endswith('__' + str(c))}} for c in range({N_CORES})]\n"
        "if 'prog' not in KK._prog_cache:\n"
        "    KK._prog_cache['prog'] = KK._build_program()\n"
        "res = KK.run_bass_kernel_spmd(KK._prog_cache['prog'], in_maps, "
        f"list(range({N_CORES})))\n"
        f"np.savez({outp!r}, "
        f"**{{str(c): res.results[c]['out'] for c in range({N_CORES})}})\n"
    )
    last = None
    for attempt in range(3):
        time.sleep(15)
        try:
            subprocess.run([sys.executable, "-c", code], check=True,
                           timeout=900)
            d = np.load(outp)
            return [d[str(c)] for c in range(N_CORES)]
        except Exception as e:  # noqa: PERF203
            last = e
    raise RuntimeError(f"subprocess retries exhausted: {last}")


# revision 14
# speedup vs baseline: 1.4020x; 1.4020x over previous
"""Bahdanau-style additive cross-attention on 8 Trainium2 NeuronCores.

Math: scores[b,q,k] = sum_h Wv[h] * tanh(qp[b,q,h] + 2*cp[b,k,h]) + bv
      out = softmax_k(scores) @ context

The tanh over the [B,Lq,Lk,H] intermediate (134M elements) is the whole
cost.  We approximate tanh(s) on |s| <= 9.2 by a truncated sine series
    tanh(s) ~= sum_r g_r * sin(w_r * s),   w_r = (r+1)*pi/L
which separates:  sin(w(a+b)) = sin(wa)cos(wb) + cos(wa)sin(wb).
So scores becomes 2R rank-128 matmuls over bf16 sin/cos factor tiles --
tensor-engine work instead of 16.8M scalar-engine tanh evals per core.

Per-frequency factors need range reduction for ACT Sin (accurate only on
[-pi, pi]): y = w*x/2pi + c, n = round(y) (int32-cast), f = y - n in
[-0.5, 0.5], then sin = Sin(2pi*f) and cos = Sin(pi/2 - 2pi*|f|)
with |f| via sign-bit masking.

Sharding: core i handles batch b = i//2, query half qh = i%2
(256 queries x 512 keys x 128 hidden per core).
"""

import os
import sys

import numpy as np

for _p in ("/opt/trn_rl_repo", "/root/.axon_site/_ro/trn_rl_repo"):
    if os.path.isdir(_p) and _p not in sys.path:
        sys.path.insert(0, _p)

import concourse.bass as bass  # noqa: E402
import concourse.mybir as mybir  # noqa: E402
import concourse.tile as tile  # noqa: E402
from concourse import bacc  # noqa: E402
from concourse.bass_utils import run_bass_kernel_spmd  # noqa: E402

AF = mybir.ActivationFunctionType
AO = mybir.AluOpType
dt = mybir.dt

# problem shape (hardcoded per contract)
B, LQ, LK = 4, 512, 512
DQ, DC, H = 256, 256, 128
N_CORES = 8
LQ_C = LQ // 2  # 256 queries per core

# sine-series fit parameters
R = 12          # number of frequencies
L = 11.0        # half-period
SMAX = 9.2      # fit domain |s| <= SMAX
K_OFF = 16.0    # positivity offset (integer) folded into prescale consts
RC = 3          # frequencies per pipeline chunk
N_CHUNKS = R // RC

TWO_PI = float(2.0 * np.pi)
HALF_PI = float(np.pi / 2.0)


def _fit_gamma() -> np.ndarray:
    s = np.linspace(0.0, SMAX, 4001)
    A = np.stack([np.sin((r + 1) * np.pi / L * s) for r in range(R)], axis=1)
    g, *_ = np.linalg.lstsq(A, np.tanh(s), rcond=None)
    return g.astype(np.float64)


_GAMMA = _fit_gamma()

_prog_cache = {}


def _register_const(nc, value, dtype=dt.float32):
    t = nc.alloc_sbuf_tensor(f"constx-{value}", [128, 1], dtype)
    nc.gpsimd.memset(t.ap(), value)
    nc.const_aps.aps[(dtype, value)] = t.ap()


def _build_program():
    nc = bacc.Bacc(None, target_bir_lowering=False)
    _register_const(nc, HALF_PI)

    # per-core inputs (host pre-transposed / pre-sharded)
    qT = nc.dram_tensor("qT", [DQ, LQ_C], dt.float32, kind="ExternalInput")
    cT = nc.dram_tensor("cT", [DC, LK], dt.float32, kind="ExternalInput")
    ctx = nc.dram_tensor("ctx", [LK, DC + 2], dt.float32, kind="ExternalInput")
    wqT = nc.dram_tensor("wqT", [DQ, H], dt.float32, kind="ExternalInput")
    wcT = nc.dram_tensor("wcT", [DC, H], dt.float32, kind="ExternalInput")
    ca = nc.dram_tensor("ca", [H, R], dt.float32, kind="ExternalInput")
    cb = nc.dram_tensor("cb", [H, R], dt.float32, kind="ExternalInput")
    gwv = nc.dram_tensor("gwv", [H, R], dt.float32, kind="ExternalInput")
    out = nc.dram_tensor("out", [LQ_C, DC], dt.float32, kind="ExternalOutput")

    FA = RC * LQ_C   # a-side chunk free size
    FB = RC * LK     # b-side chunk free size

    with tile.TileContext(nc) as tc:
        with (
            tc.tile_pool(name="sb", bufs=1) as sb,
            tc.tile_pool(name="w1", bufs=2) as w1,
            tc.tile_pool(name="w2", bufs=3) as w2,
            tc.tile_pool(name="ps_a", bufs=2, space="PSUM") as ps_a,
            tc.tile_pool(name="ps_s", bufs=4, space="PSUM") as ps_s,
        ):
            # ---- DMA in: projection operands first, ctx last ----
            wq_t = []
            for i in range(2):
                t = sb.tile([128, H], dt.float32, tag=f"wq{i}", name=f"wq{i}")
                nc.sync.dma_start(t[:], wqT[i * 128:(i + 1) * 128, :])
                wq_t.append(t)
            wc_t = []
            for i in range(2):
                t = sb.tile([128, H], dt.float32, tag=f"wc{i}", name=f"wc{i}")
                nc.sync.dma_start(t[:], wcT[i * 128:(i + 1) * 128, :])
                wc_t.append(t)
            qT_t = []
            for i in range(2):
                t = sb.tile([128, LQ_C], dt.float32, tag=f"qT{i}", name=f"qT{i}")
                nc.sync.dma_start(t[:], qT[i * 128:(i + 1) * 128, :])
                qT_t.append(t)
            cT_t = []
            for i in range(2):
                t = sb.tile([128, LK], dt.float32, tag=f"cT{i}", name=f"cT{i}")
                nc.sync.dma_start(t[:], cT[i * 128:(i + 1) * 128, :])
                cT_t.append(t)
            ca_t = sb.tile([H, R], dt.float32, tag="ca")
            nc.sync.dma_start(ca_t[:], ca[:])
            cb_t = sb.tile([H, R], dt.float32, tag="cb")
            nc.sync.dma_start(cb_t[:], cb[:])
            gwv_t = sb.tile([H, R], dt.float32, tag="gwv")
            nc.sync.dma_start(gwv_t[:], gwv[:])
            # ctx is only needed for the final attn @ context matmul
            ctx_t = []
            for i in range(4):
                t = sb.tile([128, DC + 2], dt.float32, tag=f"ctx{i}",
                            name=f"ctx{i}")
                nc.gpsimd.dma_start(t[:], ctx[i * 128:(i + 1) * 128, :])
                ctx_t.append(t)

            # ---- projections on PE (fp32): qpT[h,q], cpT[h,k] ----
            qpT_ps = ps_a.tile([H, LQ_C], dt.float32, tag="psA")
            for i in range(2):
                nc.tensor.matmul(qpT_ps[:], wq_t[i][:], qT_t[i][:],
                                 start=(i == 0), stop=(i == 1))
            cpT_ps = ps_a.tile([H, LK], dt.float32, tag="psA")
            for i in range(2):
                nc.tensor.matmul(cpT_ps[:], wc_t[i][:], cT_t[i][:],
                                 start=(i == 0), stop=(i == 1))
            qpTs = sb.tile([H, LQ_C], dt.float32, tag="qpTs")
            nc.vector.tensor_copy(qpTs[:], qpT_ps[:])
            cpTs = sb.tile([H, LK], dt.float32, tag="cpTs")
            nc.scalar.copy(cpTs[:], cpT_ps[:])

            # ---- scoresT PSUM tiles (accumulated across all chunks) ----
            scoresT = [ps_s.tile([128, LQ_C], dt.float32, tag="psS",
                                 name=f"scoresT{kt}")
                       for kt in range(4)]

            # ---- per-chunk factor pipeline + matmuls ----
            for ch in range(N_CHUNKS):
                r0 = ch * RC
                # prescale: y = x*(w_r/2pi) + table  (DVE)
                Ya = w1.tile([H, FA], dt.float32, tag="Ya", name=f"Ya{ch}")
                Yb = w1.tile([H, FB], dt.float32, tag="Yb", name=f"Yb{ch}")
                for j in range(RC):
                    r = r0 + j
                    nc.vector.tensor_scalar(
                        Ya[:, j * LQ_C:(j + 1) * LQ_C], qpTs[:],
                        float((r + 1) / (2 * L)), ca_t[:, r:r + 1],
                        AO.mult, AO.add)
                for j in range(RC):
                    r = r0 + j
                    nc.vector.tensor_scalar(
                        Yb[:, j * LK:(j + 1) * LK], cpTs[:],
                        float((r + 1) / L), cb_t[:, r:r + 1],
                        AO.mult, AO.add)
                # n = round-to-nearest(y): int32 cast on GPSIMD
                Na = w1.tile([H, FA], dt.int32, tag="Na", name=f"Na{ch}")
                nc.vector.tensor_copy(Na[:], Ya[:])
                Nb = w1.tile([H, FB], dt.int32, tag="Nb", name=f"Nb{ch}")
                nc.vector.tensor_copy(Nb[:], Yb[:])
                # f = y - n in [-0.5, 0.5]  (DVE)
                Fa = w1.tile([H, FA], dt.float32, tag="Fa", name=f"Fa{ch}")
                nc.vector.tensor_tensor(Fa[:], Ya[:], Na[:], AO.subtract)
                Fb = w1.tile([H, FB], dt.float32, tag="Fb", name=f"Fb{ch}")
                nc.vector.tensor_tensor(Fb[:], Yb[:], Nb[:], AO.subtract)
                # |f| via sign-bit mask  (DVE)
                Ga = w1.tile([H, FA], dt.float32, tag="Ga", name=f"Ga{ch}")
                nc.vector.tensor_scalar(
                    Ga[:].bitcast(dt.int32), Fa[:].bitcast(dt.int32),
                    0x7FFFFFFF, None, AO.bitwise_and)
                Gb = w1.tile([H, FB], dt.float32, tag="Gb", name=f"Gb{ch}")
                nc.vector.tensor_scalar(
                    Gb[:].bitcast(dt.int32), Fb[:].bitcast(dt.int32),
                    0x7FFFFFFF, None, AO.bitwise_and)
                # sin = Sin(2pi f); cos = Sin(pi/2 - 2pi|f|)  (ACT, bf16 out)
                sinA = w1.tile([H, FA], dt.bfloat16, tag="sinA", name=f"sinA{ch}")
                nc.scalar.activation(sinA[:], Fa[:], AF.Sin, scale=TWO_PI)
                cosA = w1.tile([H, FA], dt.bfloat16, tag="cosA", name=f"cosA{ch}")
                nc.scalar.activation(cosA[:], Ga[:], AF.Sin,
                                     bias=HALF_PI, scale=-TWO_PI)
                sinB = w2.tile([H, FB], dt.bfloat16, tag="sinB", name=f"sinB{ch}")
                nc.scalar.activation(sinB[:], Fb[:], AF.Sin, scale=TWO_PI)
                cosB = w2.tile([H, FB], dt.bfloat16, tag="cosB", name=f"cosB{ch}")
                nc.scalar.activation(cosB[:], Gb[:], AF.Sin,
                                     bias=HALF_PI, scale=-TWO_PI)
                # A-side *= g_r * Wv[h]  (DVE, bf16)
                WsinA = w2.tile([H, FA], dt.bfloat16, tag="WsinA",
                                name=f"WsinA{ch}")
                WcosA = w2.tile([H, FA], dt.bfloat16, tag="WcosA",
                                name=f"WcosA{ch}")
                for j in range(RC):
                    r = r0 + j
                    nc.vector.tensor_scalar(
                        WsinA[:, j * LQ_C:(j + 1) * LQ_C],
                        sinA[:, j * LQ_C:(j + 1) * LQ_C],
                        gwv_t[:, r:r + 1], None, AO.mult)
                    nc.vector.tensor_scalar(
                        WcosA[:, j * LQ_C:(j + 1) * LQ_C],
                        cosA[:, j * LQ_C:(j + 1) * LQ_C],
                        gwv_t[:, r:r + 1], None, AO.mult)
                # scoresT[k,q] += cosB_r^T @ WsinA_r + sinB_r^T @ WcosA_r
                first = (ch == 0)
                last = (ch == N_CHUNKS - 1)
                for kt in range(4):
                    for j in range(RC):
                        ksl = slice(j * LK + kt * 128, j * LK + kt * 128 + 128)
                        qsl = slice(j * LQ_C, (j + 1) * LQ_C)
                        nc.tensor.matmul(
                            scoresT[kt][:], cosB[:, ksl], WsinA[:, qsl],
                            start=(first and j == 0), stop=False)
                        nc.tensor.matmul(
                            scoresT[kt][:], sinB[:, ksl], WcosA[:, qsl],
                            start=False, stop=(last and j == RC - 1))

            # ---- softmax numerator: E = exp(scoresT), float32r for fast
            # final matmuls (no max-sub needed: |scores| <= sum|Wv| ~ 3.3) ----
            E_t = []
            for kt in range(4):
                e = sb.tile([128, LQ_C], dt.float32r, tag=f"E{kt}", name=f"E{kt}")
                nc.scalar.activation(e[:], scoresT[kt][:], AF.Exp)
                E_t.append(e)
            # float32r copies of context (with appended ones column:
            # column DC of the output accumulates S = sum_k E)
            ctxr_t = []
            for kt in range(4):
                t = sb.tile([128, DC + 2], dt.float32r, tag=f"ctxr{kt}",
                            name=f"ctxr{kt}")
                nc.vector.tensor_copy(t[:], ctx_t[kt][:])
                ctxr_t.append(t)

            for qt in range(2):
                qsl = slice(qt * 128, (qt + 1) * 128)
                o_ps = ps_a.tile([128, DC + 2], dt.float32, tag="psA",
                                 name=f"ops{qt}")
                for kt in range(4):
                    nc.tensor.matmul(o_ps[:], E_t[kt][:, qsl], ctxr_t[kt][:],
                                     start=(kt == 0), stop=(kt == 3))
                rec = sb.tile([128, 1], dt.float32, tag=f"rec{qt}",
                              name=f"rec{qt}")
                nc.vector.reciprocal(rec[:], o_ps[:, DC:DC + 1])
                outF = sb.tile([128, DC], dt.float32, tag=f"outF{qt}",
                               name=f"outF{qt}")
                nc.vector.tensor_scalar(outF[:], o_ps[:, :DC], rec[:], None,
                                        AO.mult)
                nc.sync.dma_start(out[qt * 128:(qt + 1) * 128, :], outF[:])

    if not nc.is_finalized():
        nc.finalize()
    return nc


def _host_tables(Wq, bq, Wc, bc, Wv):
    rr = np.arange(1, R + 1, dtype=np.float64)
    ca = (rr[None, :] * bq.astype(np.float64)[:, None] / (2 * L)) + K_OFF
    cb = (rr[None, :] * bc.astype(np.float64)[:, None] / L) + K_OFF
    gwv = _GAMMA[None, :] * Wv[0].astype(np.float64)[:, None]
    return (ca.astype(np.float32), cb.astype(np.float32),
            gwv.astype(np.float32))


def kernel(**inputs):
    query = np.ascontiguousarray(np.asarray(inputs["query"], np.float32))
    context = np.ascontiguousarray(np.asarray(inputs["context"], np.float32))
    Wq = np.asarray(inputs["Wq"], np.float32)
    bq = np.asarray(inputs["bq"], np.float32)
    Wc = np.asarray(inputs["Wc"], np.float32)
    bc = np.asarray(inputs["bc"], np.float32)
    Wv = np.asarray(inputs["Wv"], np.float32)

    if "prog" not in _prog_cache:
        _prog_cache["prog"] = _build_program()
    nc = _prog_cache["prog"]

    ca, cb, gwv = _host_tables(Wq, bq, Wc, bc, Wv)
    wqT = np.ascontiguousarray(Wq.T)  # [DQ, H]
    wcT = np.ascontiguousarray(Wc.T)  # [DC, H]

    in_maps = []
    for core in range(N_CORES):
        b, qh = core // 2, core % 2
        qTv = np.ascontiguousarray(
            query[b].T[:, qh * LQ_C:(qh + 1) * LQ_C])  # [DQ, LQ_C]
        cTv = np.ascontiguousarray(context[b].T)       # [DC, LK]
        ctx_aug = np.concatenate(
            [context[b], np.ones((LK, 1), np.float32), np.zeros((LK, 1), np.float32)], axis=1)
        in_maps.append({
            "qT": qTv, "cT": cTv, "ctx": np.ascontiguousarray(ctx_aug),
            "wqT": wqT, "wcT": wcT, "ca": ca, "cb": cb, "gwv": gwv,
        })

    try:
        res = run_bass_kernel_spmd(nc, in_maps, list(range(N_CORES)))
        per_core = [res.results[c]["out"] for c in range(N_CORES)]
    except Exception:
        # Transient NRT device crashes have been observed; the in-process
        # PJRT client can be left unusable, so retry in a fresh interpreter.
        per_core = _run_in_subprocess(in_maps)

    out = np.empty((B, LQ, DC), np.float32)
    for core in range(N_CORES):
        b, qh = core // 2, core % 2
        out[b, qh * LQ_C:(qh + 1) * LQ_C, :] = per_core[core]
    return out


def _run_in_subprocess(in_maps):
    import subprocess
    import tempfile
    import time

    tmp = tempfile.mkdtemp()
    inp = os.path.join(tmp, "in.npz")
    outp = os.path.join(tmp, "out.npz")
    flat = {}
    for c, m in enumerate(in_maps):
        for k, v in m.items():
            flat[f"{k}__{c}"] = v
    np.savez(inp, **flat)
    code = (
        "import sys, numpy as np\n"
        f"sys.path.insert(0, {os.path.dirname(os.path.abspath(__file__))!r})\n"
        "import kernel as KK\n"
        f"d = np.load({inp!r})\n"
        f"in_maps = [{{k.rsplit('__', 1)[0]: d[k] for k in d.files "
        f"if k.endswith('__' + str(c))}} for c in range({N_CORES})]\n"
        "if 'prog' not in KK._prog_cache:\n"
        "    KK._prog_cache['prog'] = KK._build_program()\n"
        "res = KK.run_bass_kernel_spmd(KK._prog_cache['prog'], in_maps, "
        f"list(range({N_CORES})))\n"
        f"np.savez({outp!r}, "
        f"**{{str(c): res.results[c]['out'] for c in range({N_CORES})}})\n"
    )
    last = None
    for attempt in range(3):
        time.sleep(15)
        try:
            subprocess.run([sys.executable, "-c", code], check=True,
                           timeout=900)
            d = np.load(outp)
            return [d[str(c)] for c in range(N_CORES)]
        except Exception as e:  # noqa: PERF203
            last = e
    raise RuntimeError(f"subprocess retries exhausted: {last}")


# revision 15
# speedup vs baseline: 1.4942x; 1.0658x over previous
"""Bahdanau-style additive cross-attention on 8 Trainium2 NeuronCores.

Math: scores[b,q,k] = sum_h Wv[h] * tanh(qp[b,q,h] + 2*cp[b,k,h]) + bv
      out = softmax_k(scores) @ context

The tanh over the [B,Lq,Lk,H] intermediate (134M elements) is the whole
cost.  We approximate tanh(s) on |s| <= 9.2 by a truncated sine series
    tanh(s) ~= sum_r g_r * sin(w_r * s),   w_r = (r+1)*pi/L
which separates:  sin(w(a+b)) = sin(wa)cos(wb) + cos(wa)sin(wb).
So scores becomes 2R rank-128 matmuls over bf16 sin/cos factor tiles --
tensor-engine work instead of 16.8M scalar-engine tanh evals per core.

Per-frequency factors need range reduction for ACT Sin (accurate only on
[-pi, pi]): y = w*x/2pi + c, n = round(y) (int32-cast), f = y - n in
[-0.5, 0.5], then sin = Sin(2pi*f) and cos = Sin(pi/2 - 2pi*|f|)
with |f| via sign-bit masking.

Sharding: core i handles batch b = i//2, query half qh = i%2
(256 queries x 512 keys x 128 hidden per core).
"""

import os
import sys

import numpy as np

for _p in ("/opt/trn_rl_repo", "/root/.axon_site/_ro/trn_rl_repo"):
    if os.path.isdir(_p) and _p not in sys.path:
        sys.path.insert(0, _p)

import concourse.bass as bass  # noqa: E402
import concourse.mybir as mybir  # noqa: E402
import concourse.tile as tile  # noqa: E402
from concourse import bacc  # noqa: E402
from concourse.bass_utils import run_bass_kernel_spmd  # noqa: E402

AF = mybir.ActivationFunctionType
AO = mybir.AluOpType
dt = mybir.dt

# problem shape (hardcoded per contract)
B, LQ, LK = 4, 512, 512
DQ, DC, H = 256, 256, 128
N_CORES = 8
LQ_C = LQ // 2  # 256 queries per core

# sine-series fit parameters
R = 12          # number of frequencies
L = 11.0        # half-period
SMAX = 9.2      # fit domain |s| <= SMAX
K_OFF = 16.0    # positivity offset (integer) folded into prescale consts
RC = 3          # frequencies per pipeline chunk
N_CHUNKS = R // RC

TWO_PI = float(2.0 * np.pi)
HALF_PI = float(np.pi / 2.0)


def _fit_gamma() -> np.ndarray:
    s = np.linspace(0.0, SMAX, 4001)
    A = np.stack([np.sin((r + 1) * np.pi / L * s) for r in range(R)], axis=1)
    g, *_ = np.linalg.lstsq(A, np.tanh(s), rcond=None)
    return g.astype(np.float64)


_GAMMA = _fit_gamma()

_prog_cache = {}


def _register_const(nc, value, dtype=dt.float32):
    t = nc.alloc_sbuf_tensor(f"constx-{value}", [128, 1], dtype)
    nc.gpsimd.memset(t.ap(), value)
    nc.const_aps.aps[(dtype, value)] = t.ap()


def _build_program():
    nc = bacc.Bacc(None, target_bir_lowering=False)
    _register_const(nc, HALF_PI)

    # per-core inputs (host pre-transposed / pre-sharded)
    qT = nc.dram_tensor("qT", [DQ, LQ_C], dt.float32, kind="ExternalInput")
    cT = nc.dram_tensor("cT", [DC, LK], dt.float32, kind="ExternalInput")
    ctx = nc.dram_tensor("ctx", [LK, DC + 2], dt.float32, kind="ExternalInput")
    wqT = nc.dram_tensor("wqT", [DQ, H], dt.float32, kind="ExternalInput")
    wcT = nc.dram_tensor("wcT", [DC, H], dt.float32, kind="ExternalInput")
    ca = nc.dram_tensor("ca", [H, R], dt.float32, kind="ExternalInput")
    cb = nc.dram_tensor("cb", [H, R], dt.float32, kind="ExternalInput")
    gwv = nc.dram_tensor("gwv", [H, R], dt.float32, kind="ExternalInput")
    out = nc.dram_tensor("out", [LQ_C, DC], dt.float32, kind="ExternalOutput")

    FA = RC * LQ_C   # a-side chunk free size
    FB = RC * LK     # b-side chunk free size

    with tile.TileContext(nc) as tc:
        with (
            tc.tile_pool(name="sb", bufs=1) as sb,
            tc.tile_pool(name="w1", bufs=2) as w1,
            tc.tile_pool(name="w2", bufs=3) as w2,
            tc.tile_pool(name="ps_a", bufs=2, space="PSUM") as ps_a,
            tc.tile_pool(name="ps_s", bufs=4, space="PSUM") as ps_s,
        ):
            # ---- DMA in: projection operands first, ctx last ----
            wq_t = []
            for i in range(2):
                t = sb.tile([128, H], dt.float32, tag=f"wq{i}", name=f"wq{i}")
                nc.sync.dma_start(t[:], wqT[i * 128:(i + 1) * 128, :])
                wq_t.append(t)
            wc_t = []
            for i in range(2):
                t = sb.tile([128, H], dt.float32, tag=f"wc{i}", name=f"wc{i}")
                nc.sync.dma_start(t[:], wcT[i * 128:(i + 1) * 128, :])
                wc_t.append(t)
            qT_t = []
            for i in range(2):
                t = sb.tile([128, LQ_C], dt.float32, tag=f"qT{i}", name=f"qT{i}")
                nc.sync.dma_start(t[:], qT[i * 128:(i + 1) * 128, :])
                qT_t.append(t)
            cT_t = []
            for i in range(2):
                t = sb.tile([128, LK], dt.float32, tag=f"cT{i}", name=f"cT{i}")
                nc.sync.dma_start(t[:], cT[i * 128:(i + 1) * 128, :])
                cT_t.append(t)
            ca_t = sb.tile([H, R], dt.float32, tag="ca")
            nc.sync.dma_start(ca_t[:], ca[:])
            cb_t = sb.tile([H, R], dt.float32, tag="cb")
            nc.sync.dma_start(cb_t[:], cb[:])
            gwv_t = sb.tile([H, R], dt.float32, tag="gwv")
            nc.sync.dma_start(gwv_t[:], gwv[:])
            # ctx is only needed for the final attn @ context matmul
            ctx_t = []
            for i in range(4):
                t = sb.tile([128, DC + 2], dt.float32, tag=f"ctx{i}",
                            name=f"ctx{i}")
                nc.gpsimd.dma_start(t[:], ctx[i * 128:(i + 1) * 128, :])
                ctx_t.append(t)

            # ---- projections on PE (fp32): qpT[h,q], cpT[h,k] ----
            qpT_ps = ps_a.tile([H, LQ_C], dt.float32, tag="psA")
            for i in range(2):
                nc.tensor.matmul(qpT_ps[:], wq_t[i][:], qT_t[i][:],
                                 start=(i == 0), stop=(i == 1))
            cpT_ps = ps_a.tile([H, LK], dt.float32, tag="psA")
            for i in range(2):
                nc.tensor.matmul(cpT_ps[:], wc_t[i][:], cT_t[i][:],
                                 start=(i == 0), stop=(i == 1))
            qpTs = sb.tile([H, LQ_C], dt.float32, tag="qpTs")
            nc.vector.tensor_copy(qpTs[:], qpT_ps[:])
            cpTs = sb.tile([H, LK], dt.float32, tag="cpTs")
            nc.scalar.copy(cpTs[:], cpT_ps[:])

            # ---- scoresT PSUM tiles (accumulated across all chunks) ----
            scoresT = [ps_s.tile([128, LQ_C], dt.float32, tag="psS",
                                 name=f"scoresT{kt}")
                       for kt in range(4)]

            # ---- per-chunk factor pipeline + matmuls ----
            for ch in range(N_CHUNKS):
                r0 = ch * RC
                # prescale: y = x*(w_r/2pi) + table  (DVE)
                Ya = w1.tile([H, FA], dt.float32, tag="Ya", name=f"Ya{ch}")
                Yb = w1.tile([H, FB], dt.float32, tag="Yb", name=f"Yb{ch}")
                for j in range(RC):
                    r = r0 + j
                    nc.vector.tensor_scalar(
                        Ya[:, j * LQ_C:(j + 1) * LQ_C], qpTs[:],
                        float((r + 1) / (2 * L)), ca_t[:, r:r + 1],
                        AO.mult, AO.add)
                for j in range(RC):
                    r = r0 + j
                    nc.scalar.activation(
                        Yb[:, j * LK:(j + 1) * LK], cpTs[:], AF.Identity,
                        bias=cb_t[:, r:r + 1], scale=float((r + 1) / L))
                # n = round-to-nearest(y): int32 cast on GPSIMD
                Na = w1.tile([H, FA], dt.int32, tag="Na", name=f"Na{ch}")
                nc.vector.tensor_copy(Na[:], Ya[:])
                Nb = w1.tile([H, FB], dt.int32, tag="Nb", name=f"Nb{ch}")
                nc.vector.tensor_copy(Nb[:], Yb[:])
                # f = y - n in [-0.5, 0.5]  (DVE)
                Fa = w1.tile([H, FA], dt.float32, tag="Fa", name=f"Fa{ch}")
                nc.vector.tensor_tensor(Fa[:], Ya[:], Na[:], AO.subtract)
                Fb = w1.tile([H, FB], dt.float32, tag="Fb", name=f"Fb{ch}")
                nc.vector.tensor_tensor(Fb[:], Yb[:], Nb[:], AO.subtract)
                # |f| via sign-bit mask  (DVE)
                Ga = w1.tile([H, FA], dt.float32, tag="Ga", name=f"Ga{ch}")
                nc.vector.tensor_scalar(
                    Ga[:].bitcast(dt.int32), Fa[:].bitcast(dt.int32),
                    0x7FFFFFFF, None, AO.bitwise_and)
                Gb = w1.tile([H, FB], dt.float32, tag="Gb", name=f"Gb{ch}")
                nc.vector.tensor_scalar(
                    Gb[:].bitcast(dt.int32), Fb[:].bitcast(dt.int32),
                    0x7FFFFFFF, None, AO.bitwise_and)
                # sin = Sin(2pi f); cos = Sin(pi/2 - 2pi|f|)  (ACT, bf16 out)
                sinA = w1.tile([H, FA], dt.bfloat16, tag="sinA", name=f"sinA{ch}")
                nc.scalar.activation(sinA[:], Fa[:], AF.Sin, scale=TWO_PI)
                cosA = w1.tile([H, FA], dt.bfloat16, tag="cosA", name=f"cosA{ch}")
                nc.scalar.activation(cosA[:], Ga[:], AF.Sin,
                                     bias=HALF_PI, scale=-TWO_PI)
                sinB = w2.tile([H, FB], dt.bfloat16, tag="sinB", name=f"sinB{ch}")
                nc.scalar.activation(sinB[:], Fb[:], AF.Sin, scale=TWO_PI)
                cosB = w2.tile([H, FB], dt.bfloat16, tag="cosB", name=f"cosB{ch}")
                nc.scalar.activation(cosB[:], Gb[:], AF.Sin,
                                     bias=HALF_PI, scale=-TWO_PI)
                # A-side *= g_r * Wv[h]  (DVE, bf16)
                WsinA = w2.tile([H, FA], dt.bfloat16, tag="WsinA",
                                name=f"WsinA{ch}")
                WcosA = w2.tile([H, FA], dt.bfloat16, tag="WcosA",
                                name=f"WcosA{ch}")
                for j in range(RC):
                    r = r0 + j
                    nc.vector.tensor_scalar(
                        WsinA[:, j * LQ_C:(j + 1) * LQ_C],
                        sinA[:, j * LQ_C:(j + 1) * LQ_C],
                        gwv_t[:, r:r + 1], None, AO.mult)
                    nc.vector.tensor_scalar(
                        WcosA[:, j * LQ_C:(j + 1) * LQ_C],
                        cosA[:, j * LQ_C:(j + 1) * LQ_C],
                        gwv_t[:, r:r + 1], None, AO.mult)
                # scoresT[k,q] += cosB_r^T @ WsinA_r + sinB_r^T @ WcosA_r
                first = (ch == 0)
                last = (ch == N_CHUNKS - 1)
                for kt in range(4):
                    for j in range(RC):
                        ksl = slice(j * LK + kt * 128, j * LK + kt * 128 + 128)
                        qsl = slice(j * LQ_C, (j + 1) * LQ_C)
                        nc.tensor.matmul(
                            scoresT[kt][:], cosB[:, ksl], WsinA[:, qsl],
                            start=(first and j == 0), stop=False)
                        nc.tensor.matmul(
                            scoresT[kt][:], sinB[:, ksl], WcosA[:, qsl],
                            start=False, stop=(last and j == RC - 1))

            # ---- softmax numerator: E = exp(scoresT), float32r for fast
            # final matmuls (no max-sub needed: |scores| <= sum|Wv| ~ 3.3) ----
            E_t = []
            for kt in range(4):
                e = sb.tile([128, LQ_C], dt.float32r, tag=f"E{kt}", name=f"E{kt}")
                nc.scalar.activation(e[:], scoresT[kt][:], AF.Exp)
                E_t.append(e)
            # float32r copies of context (with appended ones column:
            # column DC of the output accumulates S = sum_k E)
            ctxr_t = []
            for kt in range(4):
                t = sb.tile([128, DC + 2], dt.float32r, tag=f"ctxr{kt}",
                            name=f"ctxr{kt}")
                nc.vector.tensor_copy(t[:], ctx_t[kt][:])
                ctxr_t.append(t)

            for qt in range(2):
                qsl = slice(qt * 128, (qt + 1) * 128)
                o_ps = ps_a.tile([128, DC + 2], dt.float32, tag="psA",
                                 name=f"ops{qt}")
                for kt in range(4):
                    nc.tensor.matmul(o_ps[:], E_t[kt][:, qsl], ctxr_t[kt][:],
                                     start=(kt == 0), stop=(kt == 3))
                rec = sb.tile([128, 1], dt.float32, tag=f"rec{qt}",
                              name=f"rec{qt}")
                nc.vector.reciprocal(rec[:], o_ps[:, DC:DC + 1])
                outF = sb.tile([128, DC], dt.float32, tag=f"outF{qt}",
                               name=f"outF{qt}")
                nc.vector.tensor_scalar(outF[:], o_ps[:, :DC], rec[:], None,
                                        AO.mult)
                nc.sync.dma_start(out[qt * 128:(qt + 1) * 128, :], outF[:])

    if not nc.is_finalized():
        nc.finalize()
    return nc


def _host_tables(Wq, bq, Wc, bc, Wv):
    rr = np.arange(1, R + 1, dtype=np.float64)
    ca = (rr[None, :] * bq.astype(np.float64)[:, None] / (2 * L)) + K_OFF
    cb = (rr[None, :] * bc.astype(np.float64)[:, None] / L) + K_OFF
    gwv = _GAMMA[None, :] * Wv[0].astype(np.float64)[:, None]
    return (ca.astype(np.float32), cb.astype(np.float32),
            gwv.astype(np.float32))


def kernel(**inputs):
    query = np.ascontiguousarray(np.asarray(inputs["query"], np.float32))
    context = np.ascontiguousarray(np.asarray(inputs["context"], np.float32))
    Wq = np.asarray(inputs["Wq"], np.float32)
    bq = np.asarray(inputs["bq"], np.float32)
    Wc = np.asarray(inputs["Wc"], np.float32)
    bc = np.asarray(inputs["bc"], np.float32)
    Wv = np.asarray(inputs["Wv"], np.float32)

    if "prog" not in _prog_cache:
        _prog_cache["prog"] = _build_program()
    nc = _prog_cache["prog"]

    ca, cb, gwv = _host_tables(Wq, bq, Wc, bc, Wv)
    wqT = np.ascontiguousarray(Wq.T)  # [DQ, H]
    wcT = np.ascontiguousarray(Wc.T)  # [DC, H]

    in_maps = []
    for core in range(N_CORES):
        b, qh = core // 2, core % 2
        qTv = np.ascontiguousarray(
            query[b].T[:, qh * LQ_C:(qh + 1) * LQ_C])  # [DQ, LQ_C]
        cTv = np.ascontiguousarray(context[b].T)       # [DC, LK]
        ctx_aug = np.concatenate(
            [context[b], np.ones((LK, 1), np.float32), np.zeros((LK, 1), np.float32)], axis=1)
        in_maps.append({
            "qT": qTv, "cT": cTv, "ctx": np.ascontiguousarray(ctx_aug),
            "wqT": wqT, "wcT": wcT, "ca": ca, "cb": cb, "gwv": gwv,
        })

    try:
        res = run_bass_kernel_spmd(nc, in_maps, list(range(N_CORES)))
        per_core = [res.results[c]["out"] for c in range(N_CORES)]
    except Exception:
        # Transient NRT device crashes have been observed; the in-process
        # PJRT client can be left unusable, so retry in a fresh interpreter.
        per_core = _run_in_subprocess(in_maps)

    out = np.empty((B, LQ, DC), np.float32)
    for core in range(N_CORES):
        b, qh = core // 2, core % 2
        out[b, qh * LQ_C:(qh + 1) * LQ_C, :] = per_core[core]
    return out


def _run_in_subprocess(in_maps):
    import subprocess
    import tempfile
    import time

    tmp = tempfile.mkdtemp()
    inp = os.path.join(tmp, "in.npz")
    outp = os.path.join(tmp, "out.npz")
    flat = {}
    for c, m in enumerate(in_maps):
        for k, v in m.items():
            flat[f"{k}__{c}"] = v
    np.savez(inp, **flat)
    code = (
        "import sys, numpy as np\n"
        f"sys.path.insert(0, {os.path.dirname(os.path.abspath(__file__))!r})\n"
        "import kernel as KK\n"
        f"d = np.load({inp!r})\n"
        f"in_maps = [{{k.rsplit('__', 1)[0]: d[k] for k in d.files "
        f"if k.endswith('__' + str(c))}} for c in range({N_CORES})]\n"
        "if 'prog' not in KK._prog_cache:\n"
        "    KK._prog_cache['prog'] = KK._build_program()\n"
        "res = KK.run_bass_kernel_spmd(KK._prog_cache['prog'], in_maps, "
        f"list(range({N_CORES})))\n"
        f"np.savez({outp!r}, "
        f"**{{str(c): res.results[c]['out'] for c in range({N_CORES})}})\n"
    )
    last = None
    for attempt in range(3):
        time.sleep(15)
        try:
            subprocess.run([sys.executable, "-c", code], check=True,
                           timeout=900)
            d = np.load(outp)
            return [d[str(c)] for c in range(N_CORES)]
        except Exception as e:  # noqa: PERF203
            last = e
    raise RuntimeError(f"subprocess retries exhausted: {last}")
